# revision 19
# baseline (speedup 1.0000x reference)
"""Fused QKV projection + RMSNorm + RoPE + GQA repeat for Trainium2.

Reference computation (per nn_Attention_33681133535344):
    q = rope(rmsnorm(x @ Wq, gq))   -> (B, H, T, DH)
    k = rope(rmsnorm(x @ Wk, gk))   -> repeat -> (B, H, T, DH)
    v = x @ Wv                      -> repeat -> (B, H, T, DH)

Sharding: rows of flattened (B*T, D) x are split across the 8 NeuronCores
(1024 tokens each); weights are replicated. RMSNorm reduces over the full
feature dim, which is row-local under this sharding, so no collectives are
needed. Each core computes x_shard @ [Wq|Wk|Wv] as one 1024x4096x6144
bf16 matmul stream (f32 PSUM accumulation), applies RoPE at PSUM eviction
(RoPE commutes with the per-token RMS scale), accumulates sum-of-squares
from pre-rope PSUM via an ACT Square with row-sum accumulator, stages
roped-unnormalized q/k to DRAM in bf16, and applies scale*gamma in a fused
second pass that overlaps the tail of the matmul stream. Outputs land in
bf16 (host upcasts); this halves stage/output DMA so the W-slab prefetch
never starves the PE. The first 512-col slab is processed as two 256-col
half-slabs so the first PSUM group only needs 3 MB of inputs (W half-slab
+ x tile) before it can close — the DMA-bound startup crawl shortens by
~5us. The GQA head-repeat is pure duplication and is done on the host
during unsharding.
"""

import sys

sys.path.insert(0, "/opt/trn_rl_repo")

import numpy as np
import ml_dtypes

B, T, D = 2, 4096, 4096
H, HKV = 32, 8
DH = D // H  # 128
EPS = 1e-5
ROPE_BASE = 10000.0

NCORES = 8
P = 128
TLOC = (B * T) // NCORES  # 1024 tokens per core
TT = TLOC // P  # 8 token tiles per core
KO = D // P  # 32 contraction chunks
NQ = D  # 4096 q cols
NKV = HKV * DH  # 1024 k cols (same for v)
NCOLS = NQ + 2 * NKV  # 6144 fused output cols
NT = 512  # slab width == matmul moving free dim
NT2 = NT // 2  # half-slab width for the startup slab
NSLAB = NCOLS // NT  # 12 (8 q, 2 k, 2 v)
Q_SLABS = NQ // NT  # 8
K_SLABS = NKV // NT  # 2
PH2_CH = 1024  # phase-2 chunk width

BF16 = ml_dtypes.bfloat16

_CACHE = {}


def _build():
    import concourse.mybir as mybir
    import concourse.tile as tile
    from concourse import bacc

    f32 = mybir.dt.float32
    bf16 = mybir.dt.bfloat16
    mult = mybir.AluOpType.mult

    nc = bacc.Bacc("TRN2", target_bir_lowering=False, debug=False)

    # layouts chosen so every DMA is contiguous per partition row:
    # xt[tt, ki, ko, t], w[oc, ki, ko, n] (slab-major), w0 = slab 0 split
    # into two separately-contiguous 256-col halves for the startup crawl
    xt = nc.declare_dram_parameter("xt", [TT, P, KO, P], bf16, isOutput=False)
    w = nc.declare_dram_parameter("w", [NSLAB - 1, P, KO, NT], bf16, isOutput=False)
    # slab 0 split into two 256-col halves, each host-packed into standard
    # [P, KOQ, NT]-shaped tiles (two ko-chunks per 512-wide row) so they
    # reuse the wq pool buffers and keep 8KB-contiguous DMA lines
    w0 = nc.declare_dram_parameter("w0", [2, 2, P, KO // 4, NT], bf16, isOutput=False)
    cose = nc.declare_dram_parameter("cose", [P, TT, DH], f32, isOutput=False)
    sine = nc.declare_dram_parameter("sine", [P, TT, DH], f32, isOutput=False)
    grep = nc.declare_dram_parameter("grep", [P, NQ + NKV], bf16, isOutput=False)
    # outputs in bf16: halves the stage/reload/output DMA traffic (host
    # upcasts); rel-err stays ~3e-3, far under the 2e-2 gate
    q_out = nc.declare_dram_parameter("q", [TT, P, NQ], bf16, isOutput=True)
    k_out = nc.declare_dram_parameter("k", [TT, P, NKV], bf16, isOutput=True)
    v_out = nc.declare_dram_parameter("v", [TT, P, NKV], bf16, isOutput=True)

    with tile.TileContext(nc) as tc:
        with (
            tc.tile_pool(name="const", bufs=1) as const,
            tc.tile_pool(name="xp", bufs=1) as xp,
            tc.tile_pool(name="wp", bufs=2) as wp,
            tc.tile_pool(name="ev", bufs=2) as ev,
            tc.tile_pool(name="ph2", bufs=4) as ph2,
            tc.tile_pool(name="psp", bufs=6, space="PSUM") as psp,
            tc.tile_pool(name="dram", bufs=1, space="DRAM") as dram,
        ):
            KQ = 4  # k-quarters per W slab
            KOQ = KO // KQ
            wq_tiles = {}

            def load_wq(oc, q):
                t = wp.tile([P, KOQ, NT], bf16, tag=f"wq{q}")
                nc.sync.dma_start(t[:], w[oc - 1, :, q * KOQ : (q + 1) * KOQ, :])
                return t

            def load_wslab(oc):
                wq_tiles[oc] = [load_wq(oc, q) for q in range(KQ)]

            def load_w0t(h, i):
                t = wp.tile([P, KOQ, NT], bf16, tag=f"wq{2 * h + i}")
                nc.sync.dma_start(t[:], w0[h, i])
                return t

            # x tiles split into ko-halves; the startup order interleaves
            # W0-half0 quarters with x0 halves so the first group's 3 MB
            # dependency streams in as early as possible
            KOH = KO // 2
            xsb_t = [
                [
                    xp.tile([P, KOH, P], bf16, tag=f"x{tt}h{h}", name=f"xsb{tt}h{h}")
                    for h in range(2)
                ]
                for tt in range(TT)
            ]

            def load_x(tt, h):
                nc.sync.dma_start(
                    xsb_t[tt][h][:], xt[tt, :, h * KOH : (h + 1) * KOH, :]
                )

            w0h0 = [load_w0t(0, 0)]
            load_x(0, 0)
            w0h0.append(load_w0t(0, 1))
            load_x(0, 1)
            w0h1 = [load_w0t(1, 0), load_w0t(1, 1)]
            for tt in range(1, TT):
                load_x(tt, 0)
                load_x(tt, 1)
            cosb = const.tile([P, TT, DH], f32)
            nc.sync.dma_start(cosb[:], cose[:])
            sinb = const.tile([P, TT, DH], f32)
            nc.sync.dma_start(sinb[:], sine[:])
            gsb = const.tile([P, NQ + NKV], bf16)
            nc.sync.dma_start(gsb[:], grep[:])

            epsb = const.tile([P, 1], f32)
            nc.vector.memset(epsb[:], EPS)
            # HAM warm-up: matmuls on uninitialized SBUF garbage during the
            # initial input-DMA window. ~3.5us of PE activity flips the clock
            # gate to 2.4 GHz before the real stream starts; sized so the
            # warm-ups end right as the first half-slab's inputs land.
            warm_l = const.tile([P, P], bf16)
            nc.vector.memset(warm_l[:], 0.0)
            warm_r = const.tile([P, NT2], bf16)
            nc.vector.memset(warm_r[:], 0.0)
            warm_ps = psp.tile([P, NT2], f32, tag="ps0", bufs=2)
            for i in range(12):
                nc.tensor.matmul(
                    warm_ps[:], warm_l[:], warm_r[:], start=True, stop=True
                )

            statq = const.tile([P, TT], f32)
            nc.vector.memset(statq[:], 0.0)
            statk = const.tile([P, TT], f32)
            nc.vector.memset(statk[:], 0.0)
            scaleq = const.tile([P, TT], f32)
            scalek = const.tile([P, TT], f32)

            qs = dram.tile([TT, P, NQ], bf16)
            ks = dram.tile([TT, P, NKV], bf16)

            def evict(ps, tt, col0, nt, fillers):
                nh = nt // DH
                if col0 < NQ + NKV:
                    is_q = col0 < NQ
                    stats = statq if is_q else statk
                    stage = qs if is_q else ks
                    scol = col0 if is_q else col0 - NQ
                    # RoPE: out = ps * cosE + swap_pairs(ps) * sinE
                    # (sinE carries the -sin on even lanes)
                    ps4 = ps[:].rearrange("p (h j s) -> p h j s", h=nh, s=2)
                    rot = ev.tile([P, nt], f32, tag=f"rot{nt}", bufs=3)
                    rot4 = rot[:].rearrange("p (h j s) -> p h j s", h=nh, s=2)
                    nc.scalar.copy(rot4[:, :, :, 0], ps4[:, :, :, 1])
                    nc.scalar.copy(rot4[:, :, :, 1], ps4[:, :, :, 0])
                    cos_bc = cosb[:, tt, None, :].to_broadcast((P, nh, DH))
                    sin_bc = sinb[:, tt, None, :].to_broadcast((P, nh, DH))
                    st = ev.tile([P, nt], f32, tag=f"st{nt}", bufs=3)
                    st3 = st[:].rearrange("p (h d) -> p h d", h=nh)
                    ps3 = ps[:].rearrange("p (h d) -> p h d", h=nh)
                    rot3 = rot[:].rearrange("p (h d) -> p h d", h=nh)
                    nc.vector.tensor_tensor(st3, ps3, cos_bc, mult)
                    nc.vector.tensor_tensor(rot3, rot3, sin_bc, mult)
                    stb = ev.tile([P, nt], bf16, tag=f"stb{nt}", bufs=3)
                    nc.vector.tensor_add(stb[:], st[:], rot[:])
                    # per-token sum of squares of the pre-norm projection,
                    # from PSUM via ACT Square (+ per-partition row sum);
                    # tensor_tensor_reduce faults at runtime on this stack
                    sq = ev.tile([P, nt], f32, tag=f"sq{nt}", bufs=1)
                    acc = ev.tile([P, 1], f32, tag="acc")
                    nc.scalar.activation(
                        sq[:],
                        ps[:],
                        mybir.ActivationFunctionType.Square,
                        accum_out=acc[:, 0:1],
                    )
                    nc.vector.tensor_add(
                        stats[:, tt : tt + 1], stats[:, tt : tt + 1], acc[:, 0:1]
                    )
                    nc.sync.dma_start(stage[tt, :, scol : scol + nt], stb[:])
                else:
                    scol = col0 - NQ - NKV
                    vt = ev.tile([P, nt], bf16, tag="vt")
                    nc.vector.tensor_copy(vt[:], ps[:])
                    nc.sync.dma_start(v_out[tt, :, scol : scol + nt], vt[:])
                if fillers:
                    fillers.pop(0)()

            def do_group(wslice, tt, col0, nt, pstag, fillers):
                ps = psp.tile([P, nt], f32, tag=pstag, bufs=(2 if nt == NT2 else 6))
                for ko in range(KO):
                    nc.tensor.matmul(
                        ps[:],
                        xsb_t[tt][ko // KOH][:, ko % KOH, :],
                        wslice(ko),
                        start=(ko == 0),
                        stop=(ko == KO - 1),
                    )
                evict(ps, tt, col0, nt, fillers)

            def do_slab(oc, fillers=None):
                if oc not in wq_tiles:
                    load_wslab(oc)
                wsb = wq_tiles.pop(oc)
                if oc + 1 < NSLAB:
                    load_wslab(oc + 1)  # prefetch next slab
                for tt in range(TT):
                    do_group(
                        lambda ko: wsb[ko // KOQ][:, ko % KOQ, :],
                        tt, oc * NT, NT, "ps", fillers,
                    )

            def w0slice(tiles):
                # tile i holds ko [16i, 16i+16) packed two chunks per row:
                # [P, 8, 2*NT2]; chunk ko -> [:, (ko%16)//2, (ko%2)*NT2 :]
                def f(ko):
                    t = tiles[ko // (2 * KOQ)]
                    j = (ko % (2 * KOQ)) // 2
                    s = ko % 2
                    return t[:, j, s * NT2 : (s + 1) * NT2]
                return f

            # slab 0 as two 256-col half-slabs (startup crawl); prefetch of
            # slab 1 starts with half 1 so the sync queue stays input-ordered
            for tt in range(TT):
                do_group(w0slice(w0h0), tt, 0, NT2, "ps0", None)
            load_wslab(1)
            for tt in range(TT):
                do_group(w0slice(w0h1), tt, NT2, NT2, "ps0", None)

            # slabs 1..7 = rest of q, 8..9 = k, 10..11 = v. Phase-2
            # (scale*gamma on the staged roped projections) is interleaved one
            # half-token-tile per matmul group across slabs 8..10 so its
            # DVE/DMA load never bursts; slab 11 runs clean for a short tail.
            for oc in range(1, Q_SLABS):
                do_slab(oc)

            def phase2_scale(stats, scale_tile, nd):
                # scale = 1 / sqrt(ssq/nd + eps)
                nc.scalar.activation(
                    scale_tile[:],
                    stats[:],
                    mybir.ActivationFunctionType.Sqrt,
                    bias=epsb[:, 0:1],
                    scale=1.0 / nd,
                )
                nc.vector.reciprocal(scale_tile[:], scale_tile[:])

            def phase2_chunks(stage, scale_tile, goff, out_ext, tt, c0s):
                # phase-2 DMAs ride the (idle) GpSimd queue so they can't
                # delay W-slab prefetch issues on the Sync queue
                for c0 in c0s:
                    t2 = ph2.tile([P, PH2_CH], bf16, tag="p2")
                    nc.gpsimd.dma_start(t2[:], stage[tt, :, c0 : c0 + PH2_CH])
                    nc.vector.scalar_tensor_tensor(
                        out=t2[:],
                        in0=t2[:],
                        scalar=scale_tile[:, tt : tt + 1],
                        in1=gsb[:, goff + c0 : goff + c0 + PH2_CH],
                        op0=mult,
                        op1=mult,
                    )
                    nc.gpsimd.dma_start(out_ext[tt, :, c0 : c0 + PH2_CH], t2[:])

            def p2_filler(stage, scale_tile, goff, out_ext, tt, c0s):
                return lambda: phase2_chunks(stage, scale_tile, goff, out_ext, tt, c0s)

            phase2_scale(statq, scaleq, NQ)
            qf = [
                p2_filler(qs, scaleq, 0, q_out, tt,
                          range(h * PH2_CH * 2, (h + 1) * PH2_CH * 2, PH2_CH))
                for tt in range(TT)
                for h in range(2)
            ]
            do_slab(Q_SLABS, fillers=qf[:TT])
            do_slab(Q_SLABS + 1, fillers=qf[TT:])
            phase2_scale(statk, scalek, NKV)
            kf = [
                p2_filler(ks, scalek, NQ, k_out, tt, range(0, NKV, PH2_CH))
                for tt in range(TT)
            ]
            do_slab(Q_SLABS + K_SLABS, fillers=kf)
            do_slab(Q_SLABS + K_SLABS + 1)

    nc.compile()
    return nc


def _in_maps(x, Wq, Wk, Wv, gq, gk):
    Wcat = np.concatenate([Wq, Wk, Wv], axis=1)  # (D, NCOLS)
    # [NSLAB, P, KO, NT]: slab-major, contiguous per (slab, partition) row
    w_arr = np.ascontiguousarray(
        Wcat.reshape(KO, P, NSLAB, NT).transpose(2, 1, 0, 3)
    ).astype(BF16)
    # w0[h, i, p, j, s*NT2:(s+1)*NT2] = slab-0 cols [h*NT2:(h+1)*NT2] of
    # ko chunk 16i + 2j + s
    s0 = w_arr[0]  # [P, KO, NT]
    w0_arr = np.empty((2, 2, P, KO // 4, NT), dtype=BF16)
    for h in range(2):
        for i in range(2):
            blk = s0[:, 16 * i : 16 * (i + 1), h * NT2 : (h + 1) * NT2]
            # [P, 16, NT2] -> [P, 8, 2, NT2] -> [P, 8, NT]
            w0_arr[h, i] = blk.reshape(P, KO // 4, 2 * NT2)
    w0_arr = np.ascontiguousarray(w0_arr)
    w_rest = np.ascontiguousarray(w_arr[1:])
    g_rep = np.ascontiguousarray(
        np.tile(np.concatenate([gq, gk])[None, :], (P, 1))
    ).astype(BF16)

    xflat = np.ascontiguousarray(x.reshape(B * T, D))

    inv = 1.0 / (ROPE_BASE ** (np.arange(0, DH, 2, dtype=np.float32) / DH))
    inv = inv.astype(np.float32)

    maps = []
    for c in range(NCORES):
        rows = xflat[c * TLOC : (c + 1) * TLOC]  # (TLOC, D)
        # [TT, P, KO, P]: xt[tt, ki, ko, t] = rows[tt*P + t, ko*P + ki]
        xt = np.ascontiguousarray(
            rows.T.reshape(KO, P, TT, P).transpose(2, 1, 0, 3)
        ).astype(BF16)
        t0 = (c % (T // TLOC)) * TLOC
        t_abs = np.arange(t0, t0 + TLOC, dtype=np.float32)
        ang = t_abs[:, None] * inv[None, :]  # (TLOC, DH/2)
        cos = np.cos(ang).astype(np.float32)
        sin = np.sin(ang).astype(np.float32)
        cosE = np.repeat(cos, 2, axis=1)  # (TLOC, DH)
        sinE = np.stack([-sin, sin], axis=-1).reshape(TLOC, DH)
        cos_arr = np.ascontiguousarray(cosE.reshape(TT, P, DH).transpose(1, 0, 2))
        sin_arr = np.ascontiguousarray(
            sinE.reshape(TT, P, DH).transpose(1, 0, 2)
        ).astype(np.float32)
        maps.append(
            {"xt": xt, "w": w_rest, "w0": w0_arr, "cose": cos_arr,
             "sine": sin_arr, "grep": g_rep}
        )
    return maps


def _assemble(results):
    q = np.empty((B * T, NQ), np.float32)
    k = np.empty((B * T, NKV), np.float32)
    v = np.empty((B * T, NKV), np.float32)
    for c in range(NCORES):
        q[c * TLOC : (c + 1) * TLOC] = results[c]["q"].reshape(TLOC, NQ).astype(np.float32)
        k[c * TLOC : (c + 1) * TLOC] = results[c]["k"].reshape(TLOC, NKV).astype(np.float32)
        v[c * TLOC : (c + 1) * TLOC] = results[c]["v"].reshape(TLOC, NKV).astype(np.float32)
    q = np.ascontiguousarray(q.reshape(B, T, H, DH).transpose(0, 2, 1, 3))
    k = k.reshape(B, T, HKV, DH).transpose(0, 2, 1, 3)
    v = v.reshape(B, T, HKV, DH).transpose(0, 2, 1, 3)
    n_rep = H // HKV
    k = np.repeat(k, n_rep, axis=1)
    v = np.repeat(v, n_rep, axis=1)
    return q, k, v


def run(inputs, trace=False, trace_cores=None):
    from concourse.bass_utils import run_bass_kernel_spmd

    x = np.asarray(inputs["x"], dtype=np.float32)
    Wq = np.asarray(inputs["Wq"], dtype=np.float32)
    Wk = np.asarray(inputs["Wk"], dtype=np.float32)
    Wv = np.asarray(inputs["Wv"], dtype=np.float32)
    gq = np.asarray(inputs["gq"], dtype=np.float32)
    gk = np.asarray(inputs["gk"], dtype=np.float32)

    if "nc" not in _CACHE:
        _CACHE["nc"] = _build()
    nc = _CACHE["nc"]

    maps = _in_maps(x, Wq, Wk, Wv, gq, gk)
    res = run_bass_kernel_spmd(
        nc, maps, core_ids=list(range(NCORES)), trace=trace, trace_cores=trace_cores
    )
    out = _assemble(res.results)
    return out, res


def kernel(**inputs):
    out, _ = run(inputs, trace=False)
    return out
